# revision 1
# baseline (speedup 1.0000x reference)
"""DIF (dual-softmax) attention layer on 8 Trainium2 NeuronCores.

Sharding: core = (batch b, head-stack s), b in 0..3, s in 0..1.
Each core computes, for its batch and its 4 heads, the full dual-softmax
attention over all T rows, producing a partial output projection (sum over
its 4 heads; bias folded into stack 0). Host sums the two stack partials.

On-chip layout, "transposed-PV" design. The cost model charges a matmul by
its output free size only, so PV runs with q on the OUTPUT PARTITIONS:
  - Q^T, K^T per branch: [128 (4h x 32d), T] bf16; attention scale * log2(e)
    folded into the Q projection weights (exp runs in base 2).
  - scores: per (k-tile, hp) event, 2 row-packed matmuls (tile_position=
    (32h,0)) into one [128, 1024] PSUM tile (two heads in different banks).
  - exp: ScalarE activation OR the 2-pass custom-DVE exp (poly + ^32),
    chosen per event by a greedy busy-balance; diagonal tiles restrict to
    the valid q range and mask the 128-col triangle (DVE: in-op Idx mask;
    ACT: in-place multiply with the 0/1 triangle on GPSIMD/DVE).
  - PV+denominator: per (event, hl, 128-q-chunk) ONE matmul
      out[128 q, 33] += ex[:, hl, chunk].T @ v33[:, kt, h, :]
    where v33 carries V^T and a ones-column (col 32) so the softmax
    denominator accumulates as column 32 of the same PSUM tile. One
    start/stop per PSUM bank per (branch, q-block).
  - normalize: reciprocal of the dn column, broadcast_to multiply -> onT
    [128 q, (chunk, h, d)] bf16; DMA-transpose each [128,128] chunk back to
    [128 (h,d), 512 q] for the output projection.
  - output projection: per 128-q slice, woc/wok matmuls accumulate both
    branches into one PSUM bank; bias add; DMA out.
"""

import numpy as np
import ml_dtypes

import concourse.bass as bass
import concourse.tile as tile
from concourse import bacc, mybir, dve_ops
from concourse.dve_spec import (Spec, Src0, C0, C1, C2, C3, One, Idx,
                                lower, _spill_c3_to_src1, _has_src1 as has_src1)
from concourse.dve_uop import DveOpSpec
from concourse.bass_utils import run_bass_kernel_spmd

B, D, H, HD = 4, 256, 8, 32
HPS = 4  # heads per stack (per core)
LOG2E = 1.4426950408889634
LN2 = 0.6931471805599453
QT = 512  # q-tile width
KT = 128  # k-tile width
NCH = QT // KT  # 128-q chunks per q-block
KEXP = 32  # exp2 split factor: exp2(y) = p(y/KEXP)^KEXP on the DVE path

# minimax coefficients for p(z) = 1 + z(a + z(b + z(c + d z))) ~ 2^z, |z|<=0.5
PA, PB, PC, PD = 0.693128038, 0.24023678, 0.055870371, 0.009590248

BF16 = mybir.dt.bfloat16
F32 = mybir.dt.float32
AF = mybir.ActivationFunctionType
OP = mybir.AluOpType

_prog_cache: dict = {}


def _register_dve_op(name, spec, subdim=False):
    """Register a custom DVE op at import time, self-pinning its uops sha."""
    for op in dve_ops.OPS:
        if op.name == name:
            return op
    row = dve_ops._CUSTOM_DVE_ROW_BASE + len(dve_ops.OPS)
    shas = {}
    for ver in ("v3", "v4"):
        s = DveOpSpec(name=name, opcode=row, uops=lower(spec, ver=ver),
                      rd1_en=has_src1(spec))
        shas[ver] = s.sha(ver)
    op = dve_ops.DveOp(name, spec, subdim=subdim, uops_sha=shas)
    dve_ops.OPS.append(op)
    dve_ops._SUB_OPCODE_FOR_NAME[name] = row
    dve_ops.CUSTOM_DVE_SPECS[name] = spec
    return op


def _make_exp_ops():
    z = Src0
    poly = One + z * (C0 + z * (C1 + z * (C2 + C3 * z)))
    k1 = _register_dve_op("ANT_EXP2_POLY", Spec(
        body=_spill_c3_to_src1(poly),
        reference=lambda in0, in1, s0, s1, imm2:
            1.0 + in0 * (s0 + in0 * (s1 + in0 * (
                imm2 + np.reshape(in1, (-1,) + (1,) * (in0.ndim - 1)) * in0))),
    ))
    p = Src0
    for _ in range(5):
        p = p * p
    k2m = _register_dve_op("ANT_EXP2_SQ5M", Spec(
        body=p * (Idx >= C0),
        reference=lambda in0, in1, s0, s1, imm2:
            (in0.astype(np.float32) ** 32)
            * (np.arange(in0.shape[-1], dtype=np.float32)
               >= np.reshape(s0, (-1,) + (1,) * (in0.ndim - 1))),
    ))
    p = Src0
    for _ in range(5):
        p = p * p
    k2 = _register_dve_op("ANT_EXP2_SQ5", Spec(
        body=p,
        reference=lambda in0, in1, s0, s1, imm2: in0.astype(np.float32) ** 32,
    ))
    return k1, k2m, k2


EXP2P, EXP2SQ5M, EXP2SQ5 = _make_exp_ops()


class _Busy:
    """Build-time engine busy estimates for greedy assignment (ns)."""

    def __init__(self):
        self.t = {"act": 0.0, "dve": 0.0, "pool": 0.0}

    def pick(self, costs):
        """costs: dict engine->ns; returns engine minimizing busy+cost."""
        e = min(costs, key=lambda k: self.t[k] + costs[k])
        self.t[e] += costs[e]
        return e

    def add(self, engine, ns):
        self.t[engine] += ns


def _build_program(T, causal=True):
    nc = bacc.Bacc("TRN2", target_bir_lowering=False, debug=False)

    xc = nc.dram_tensor("xc", [2, 128, T], BF16, kind="ExternalInput")
    xk = nc.dram_tensor("xk", [2, 128, T], BF16, kind="ExternalInput")
    wall = nc.dram_tensor("wall", [2, 128, 640], BF16, kind="ExternalInput")
    wo2 = nc.dram_tensor("wo2", [128, 512], BF16, kind="ExternalInput")
    bo_r = nc.dram_tensor("bo_r", [1, 256], BF16, kind="ExternalInput")
    pio = nc.dram_tensor("pio", [128, 1], F32, kind="ExternalInput")
    mzro = nc.dram_tensor("mzro", [128, 2 * KT], BF16, kind="ExternalInput")
    y = nc.dram_tensor("y", [T, 256], F32, kind="ExternalOutput")

    NQT = T // QT
    NTT = T // KT
    bz = _Busy()

    with tile.TileContext(nc) as tc:
        with (
            tc.tile_pool(name="xin", bufs=1) as xin,
            tc.tile_pool(name="wts", bufs=1) as wts,
            tc.tile_pool(name="proj", bufs=1) as proj,
            tc.tile_pool(name="exps", bufs=4) as exps,
            tc.tile_pool(name="ex1p", bufs=3) as ex1p,
            tc.tile_pool(name="onrm", bufs=2) as onrm,
            tc.tile_pool(name="ontp", bufs=2) as ontp,
            tc.tile_pool(name="recp", bufs=4) as recp,
            tc.tile_pool(name="yout", bufs=3) as yout,
            tc.tile_pool(name="ps_sc", bufs=2, space="PSUM") as ps_sc,
            tc.tile_pool(name="ps_pv", bufs=1, space="PSUM") as ps_pv,
            tc.tile_pool(name="ps_y", bufs=2, space="PSUM") as ps_y,
        ):
            # ---- load inputs: weights first so projections start early ----
            w_sb = []
            for j in range(2):
                t = wts.tile([128, 640], BF16, tag=f"w{j}")
                nc.sync.dma_start(t[:], wall[j])
                w_sb.append(t)

            xc0 = xin.tile([128, T], BF16, tag="xc0")
            xc1 = xin.tile([128, T], BF16, tag="xc1")
            xk0 = xin.tile([128, T], BF16, tag="xk0")
            xk1 = xin.tile([128, T], BF16, tag="xk1")
            nch = 2 if T >= 1024 else 1
            for ch in range(nch):
                sl = bass.ts(ch, T // nch)
                nc.sync.dma_start(xc0[:, sl], xc[0][:, sl])
                nc.sync.dma_start(xc1[:, sl], xc[1][:, sl])
                nc.sync.dma_start(xk0[:, sl], xk[0][:, sl])
                nc.sync.dma_start(xk1[:, sl], xk[1][:, sl])
                if ch == 0:
                    wo2_sb = wts.tile([128, 512], BF16, tag="wo2")
                    bo_sb = wts.tile([1, 256], BF16, tag="bo")
                    pio_sb = wts.tile([128, 1], F32, tag="pio")
                    mzro_sb = wts.tile([128, 2 * KT], BF16, tag="mzro")
                    nc.sync.dma_start(wo2_sb[:], wo2[:])
                    nc.sync.dma_start(bo_sb[:], bo_r[:])
                    nc.sync.dma_start(pio_sb[:], pio[:])
                    nc.sync.dma_start(mzro_sb[:], mzro[:])

            # warm the ACT exp table while DMAs stream in
            warm = wts.tile([128, 1], F32, tag="warm")
            nc.vector.memset(warm[:], 0.0)
            nc.scalar.activation(warm[:], warm[:], AF.Exp, scale=1.0)

            dco_sb = wts.tile([128, 1], F32, tag="dco")
            nc.vector.memset(dco_sb[:], PD)
            ones1 = wts.tile([1, 128], BF16, tag="ones1")
            nc.vector.memset(ones1[:], 1.0)

            # ---- projections ----
            # Q^T/K^T: out[i, t] = sum_j W.T[j, i] * x^T[j, t]
            q_c = proj.tile([128, T], BF16, tag="p_qc")
            k_c = proj.tile([128, T], BF16, tag="p_kc")
            q_k = proj.tile([128, T], BF16, tag="p_qk")
            k_k = proj.tile([128, T], BF16, tag="p_kk")
            # V with ones column: [128 t, kt, h, 33]
            v33 = proj.tile([128, NTT, HPS, 33], BF16, tag="p_v")
            nc.vector.memset(v33[:, :, :, 32:33], 1.0)
            bz.add("dve", 180.0)

            qk_list = [(0, (xc0, xc1), q_c), (1, (xc0, xc1), k_c),
                       (2, (xk0, xk1), q_k), (3, (xk0, xk1), k_k)]
            for ch in range(2):
                for nm_i, xs, dst in qk_list:
                    wsl = bass.ts(nm_i, 128)
                    for nt in range(2 * ch, 2 * ch + 2):
                        ps = ps_y.tile([128, 512], F32, tag="mm")
                        sl = bass.ts(nt, 512)
                        nc.tensor.matmul(ps[:], w_sb[0][:, wsl], xs[0][:, sl],
                                         start=True, stop=False)
                        nc.tensor.matmul(ps[:], w_sb[1][:, wsl], xs[1][:, sl],
                                         start=False, stop=True)
                        eng = bz.pick({"act": 612.0, "dve": 658.0})
                        if eng == "act":
                            nc.scalar.copy(dst[:, sl], ps[:])
                        else:
                            nc.vector.tensor_copy(dst[:, sl], ps[:])
                # V: out[t, i] = sum_j x^T[j, t] * Wv.T[j, i]; 4 t-tiles/bank
                for g in range(2 * ch, 2 * ch + 2):
                    ps = ps_y.tile([128, 512], F32, tag="mm")
                    for i in range(4):
                        tt = 4 * g + i
                        sl = bass.ts(tt, KT)
                        osl = bass.ts(i, 128)
                        nc.tensor.matmul(ps[:, osl], xc0[:, sl],
                                         w_sb[0][:, 512:640],
                                         start=(i == 0), stop=False,
                                         skip_group_check=True)
                        nc.tensor.matmul(ps[:, osl], xc1[:, sl],
                                         w_sb[1][:, 512:640],
                                         start=False, stop=(i == 3),
                                         skip_group_check=True)
                    psv = ps[:].rearrange("p (t c) -> p t c", t=4)
                    psv = psv.rearrange("p t (h d) -> p t h d", h=HPS)
                    eng = bz.pick({"act": 612.0, "dve": 658.0})
                    if eng == "act":
                        nc.scalar.copy(v33[:, 4 * g:4 * g + 4, :, 0:32], psv)
                    else:
                        nc.vector.tensor_copy(v33[:, 4 * g:4 * g + 4, :, 0:32], psv)

            # ---- attention ----
            for qt in range(NQT):
                q0 = qt * QT
                nkt = (q0 + QT) // KT if causal else NTT
                ndiag = 4 if causal else 0
                on_tiles = {}
                for br, (qsb, ksb) in [("c", (q_c, k_c)), ("k", (q_k, k_k))]:
                    pv0 = ps_pv.tile([128, NCH, 2, 33], F32, tag="pv0")
                    pv1 = ps_pv.tile([128, NCH, 2, 33], F32, tag="pv1")
                    pv = [pv0, pv1]
                    started = [False, False]
                    # local per-branch balance so the exp stream alternates
                    # engines instead of forming long same-engine runs; the
                    # normalize tail is pre-charged to DVE.
                    loc = {"act": 0.0, "dve": 1400.0}
                    nev = 2 * nkt
                    iev = 0
                    for kt in range(nkt):
                        k0 = kt * KT
                        di = kt - (nkt - ndiag) if causal else -1
                        qlo = 128 * di if di > 0 else 0
                        w = 2 * (QT - qlo)
                        for hp in range(2):
                            sp = ps_sc.tile([128, 2 * QT], F32, tag="sc")
                            for hl in range(2):
                                h = 2 * hp + hl
                                nc.tensor.matmul(
                                    sp[:, QT * hl + qlo: QT * (hl + 1)],
                                    ksb[32 * h:32 * h + 32, k0:k0 + KT],
                                    qsb[32 * h:32 * h + 32, q0 + qlo:q0 + QT],
                                    start=True, stop=True,
                                    tile_position=(32 * h, 0),
                                    skip_group_check=True,
                                )
                            ex = exps.tile([128, 2, QT], BF16, tag="ex")
                            spv = sp[:].rearrange("p (l q) -> p l q", l=2)
                            c_act = 0.833 * w + 185.0
                            c_dve = 2.083 * w + 245.0
                            if nev - iev <= 3:
                                # keep the branch tail off the slow engine
                                c_dve *= 2.0
                            iev += 1
                            costs = {"act": c_act + (80.0 if di >= 0 else 0.0),
                                     "dve": c_dve}
                            eng = min(costs, key=lambda k: loc[k] + costs[k])
                            loc[eng] += costs[eng]
                            bz.add(eng, costs[eng])
                            if eng == "act":
                                nc.scalar.activation(ex[:, :, qlo:],
                                                     spv[:, :, qlo:],
                                                     AF.Exp, scale=KEXP * LN2)
                                if di >= 0:
                                    exv = ex[:, :, qlo:qlo + KT]
                                    mzv = mzro_sb[:].rearrange(
                                        "p (l q) -> p l q", l=2)
                                    meng = bz.pick({"pool": 603.0,
                                                    "dve": 340.0})
                                    if meng == "pool":
                                        nc.gpsimd.tensor_tensor(
                                            exv, exv, mzv, OP.mult)
                                    else:
                                        nc.vector.tensor_tensor(
                                            exv, exv, mzv, OP.mult)
                            else:
                                e1 = ex1p.tile([128, 2, QT], F32, tag="e1")
                                nc.vector._custom_dve(
                                    EXP2P, out=e1[:, :, qlo:],
                                    in0=spv[:, :, qlo:],
                                    in1=dco_sb[:, 0:1], s0=PA, s1=PB, imm2=PC)
                                if di >= 0:
                                    for hl in range(2):
                                        nc.vector._custom_dve(
                                            EXP2SQ5M,
                                            out=ex[:, hl, qlo:],
                                            in0=e1[:, hl, qlo:],
                                            s0=pio_sb[:, 0:1])
                                else:
                                    nc.vector._custom_dve(
                                        EXP2SQ5, out=ex[:], in0=e1[:])
                            # PV + denominator, transposed: q on out partitions
                            clo = di if di > 0 else 0
                            for hl in range(2):
                                h = 2 * hp + hl
                                for c in range(clo, NCH):
                                    st = not started[hp]
                                    started[hp] = True
                                    last = (kt == (nkt - ndiag + c if causal
                                                   else nkt - 1))
                                    stop = (last and hl == 1 and c == NCH - 1
                                            and kt == nkt - 1)
                                    nc.tensor.matmul(
                                        pv[hp][:, c, hl, :],
                                        ex[:, hl, KT * c:KT * (c + 1)],
                                        v33[:, kt, h, :],
                                        start=st, stop=stop,
                                        skip_group_check=True,
                                    )

                    # normalize: rec of dn column, broadcast multiply
                    onT = onrm.tile([128, NCH, HPS, 32], BF16, tag=f"on{br}")
                    for hp in range(2):
                        rec = recp.tile([128, NCH, 2, 1], F32, tag="rec")
                        nc.vector.reciprocal_approx_fast(
                            rec[:].rearrange("p a b c -> p (a b c)"),
                            pv[hp][:, :, :, 32:33].rearrange(
                                "p a b c -> p (a b c)"))
                        nc.vector.tensor_tensor(
                            onT[:, :, 2 * hp:2 * hp + 2, :],
                            pv[hp][:, :, :, 0:32],
                            rec[:].broadcast_to([128, NCH, 2, 32]),
                            OP.mult)
                        bz.add("dve", 700.0)
                    # transpose back to [128 (h,d), 512 q] via DMA xbar
                    on = ontp.tile([128, QT], BF16, tag=f"ot{br}")
                    for c in range(NCH):
                        nc.sync.dma_start_transpose(
                            on[:, KT * c:KT * (c + 1)],
                            onT[:, c, :, :])
                    on_tiles[br] = on

                    # output projection: this branch's half right away; the
                    # bias rides the psum chain as a rank-1 matmul.
                    for m in range(QT // 128):
                        ysl = slice(256 * (m % 2), 256 * (m % 2) + 256)
                        if br == "c":
                            if m % 2 == 0:
                                yp = ps_y.tile([128, 512], F32, tag="mm")
                                on_tiles[("yp", m // 2)] = yp
                            yp = on_tiles[("yp", m // 2)]
                            nc.tensor.matmul(
                                yp[:, ysl], ones1[:], bo_sb[:],
                                start=(m % 2 == 0), stop=False,
                                skip_group_check=True)
                            nc.tensor.matmul(
                                yp[:, ysl],
                                on[:, bass.ts(m, 128)], wo2_sb[:, 0:256],
                                start=False, stop=False,
                                skip_group_check=True)
                        else:
                            yp = on_tiles[("yp", m // 2)]
                            nc.tensor.matmul(
                                yp[:, ysl],
                                on[:, bass.ts(m, 128)], wo2_sb[:, 256:512],
                                start=False, stop=(m % 2 == 1),
                                skip_group_check=True)
                            ysb = yout.tile([128, 256], F32, tag="y")
                            ceng = bz.pick({"act": 360.0, "dve": 390.0})
                            if ceng == "act":
                                nc.scalar.copy(ysb[:], yp[:, ysl])
                            else:
                                nc.vector.tensor_copy(ysb[:], yp[:, ysl])
                            nc.sync.dma_start(
                                y[q0 + m * 128:q0 + (m + 1) * 128, :], ysb[:])

    nc.compile()
    return nc


def _bf(x):
    return np.ascontiguousarray(np.asarray(x, np.float32)).astype(ml_dtypes.bfloat16)


def _host_prep(inputs, T):
    content = np.asarray(inputs["content"], np.float32)
    category = np.asarray(inputs["category"], np.float32)
    Wqc = np.asarray(inputs["Wqc"], np.float32)
    Wkc = np.asarray(inputs["Wkc"], np.float32)
    Wv = np.asarray(inputs["Wv"], np.float32)
    Wqk = np.asarray(inputs["Wqk"], np.float32)
    Wkk = np.asarray(inputs["Wkk"], np.float32)
    Wo = np.asarray(inputs["Wo"], np.float32)
    bo = np.asarray(inputs["bo"], np.float32)
    alpha = 1.0 / (1.0 + np.exp(-float(np.asarray(inputs["alpha_logit"]))))
    nb = content.shape[0]

    scale_q = (HD ** -0.5) * LOG2E / KEXP

    pio = np.arange(128, dtype=np.float32)[:, None]
    p_idx = np.arange(128)[:, None]
    qcol = np.arange(KT)[None, :]
    mzro = np.tile((qcol >= p_idx).astype(np.float32), (1, 2))
    mzro = _bf(mzro)

    in_maps = []
    for core in range(2 * nb):
        b, s = core // 2, core % 2
        # wall[j] = [Wqc | Wkc | Wqk | Wkk | Wv].T chunks for x-chan half j
        wj = []
        for j in range(2):
            cols = []
            for W, sc in [(Wqc, scale_q), (Wkc, 1.0), (Wqk, scale_q),
                          (Wkk, 1.0), (Wv, 1.0)]:
                cols.append((W.T * sc)[128 * j:128 * (j + 1),
                                       128 * s:128 * (s + 1)])
            wj.append(np.concatenate(cols, axis=1))
        wall = _bf(np.stack(wj))
        wo2 = _bf(np.concatenate(
            [Wo.T[128 * s:128 * (s + 1), :] * (1.0 - alpha),
             Wo.T[128 * s:128 * (s + 1), :] * alpha], axis=1))
        m = {
            "xc": _bf(content[b].T.reshape(2, 128, T)),
            "xk": _bf(category[b].T.reshape(2, 128, T)),
            "wall": wall,
            "wo2": wo2,
            "bo_r": (_bf(bo[None, :]) if s == 0
                     else np.zeros((1, 256), ml_dtypes.bfloat16)),
            "pio": pio,
            "mzro": mzro,
        }
        in_maps.append(m)
    return in_maps


def _check_mask(mask, T):
    exp = np.triu(np.ones((T, T), dtype=bool), k=1)
    return np.array_equal(np.asarray(mask), exp)


def run(inputs, T=2048, cores=None, causal=True, **run_kwargs):
    """Build/compile (cached), run on hardware, return BassKernelResults."""
    key = (T, causal)
    if key not in _prog_cache:
        _prog_cache[key] = _build_program(T, causal=causal)
    nc = _prog_cache[key]
    in_maps = _host_prep(inputs, T)
    if cores is None:
        cores = list(range(len(in_maps)))
    res = run_bass_kernel_spmd(nc, [in_maps[c] for c in cores],
                               core_ids=list(range(len(cores))), **run_kwargs)
    return res


def kernel(**inputs):
    T = 2048
    mask = np.asarray(inputs["causal_mask"])
    if _check_mask(mask, T):
        causal = True
    elif not mask.any():
        causal = False
    else:
        raise NotImplementedError("kernel supports causal or empty masks only")
    res = run(inputs, T=T, causal=causal)
    nb = np.asarray(inputs["content"]).shape[0]
    out = np.empty((nb, T, D), np.float32)
    for b in range(nb):
        out[b] = res.results[2 * b]["y"] + res.results[2 * b + 1]["y"]
    return out



# revision 14
# speedup vs baseline: 1.2437x; 1.2437x over previous
"""DIF (dual-softmax) attention layer on 8 Trainium2 NeuronCores.

Sharding: core = (batch b, head-stack s), b in 0..3, s in 0..1.
Each core computes, for its batch and its 4 heads, the full dual-softmax
attention over all T rows, producing a partial output projection (sum over
its 4 heads; bias folded into stack 0). Host sums the two stack partials.

On-chip layout, "transposed-PV" design with a unified software-pipelined
event stream. The cost model charges a matmul by its output free size only,
so PV runs with q on the OUTPUT PARTITIONS:
  - Q^T, K^T per branch: [128 (4h x 32d), T] bf16; attention scale * log2(e)
    folded into the Q projection weights (exp runs in base 2).
  - events: one per (q-block, branch, k-tile, head-pair). Per event, 2
    row-packed score matmuls (tile_position=(32h,0)) into one [128, 1024]
    PSUM tile, then exp on ScalarE (AF.Exp) OR the 2-pass custom-DVE exp
    (poly + ^32), chosen by a greedy busy-balance. Diagonal tiles restrict
    to the valid q range; the 128-col triangle mask is in-op (DVE Idx) or an
    external multiply (Pool/DVE) for the ACT path.
  - PV+denominator: deferred by LAG events (so the in-order PE queue never
    blocks upcoming score matmuls behind a PV waiting on a slow exp). Per
    (event, hl, 128-q-chunk) ONE matmul
      out[128 q, 33] += ex[:, hl, chunk].T @ v33[:, kt, h, :]
    where v33 carries V^T and a ones-column (col 32) so the softmax
    denominator accumulates as column 32 of the same PSUM tile.
  - PSUM tetris (8 banks): score/proj/out-proj tiles all rotate through one
    3-deep [128,1024] f32 pool (6 banks); pv0/pv1 keep one bank each.
  - boundary work (normalize, DMA transposes back to [128 (h,d), q], output
    projection) is injected at fixed offsets into the NEXT segment's event
    stream so it overlaps and never head-blocks an engine queue.
  - QKV projection groups are issued per segment on demand so attention
    starts as soon as the first x chunks land; PSUM->SBUF copies run on
    Pool; y stores go out on the gpsimd SWDGE queue.
"""

import numpy as np
import ml_dtypes

import concourse.bass as bass
import concourse.tile as tile
from concourse import bacc, mybir, dve_ops
from concourse.dve_spec import (Spec, Src0, C0, C1, C2, C3, One, Idx,
                                lower, _spill_c3_to_src1, _has_src1 as has_src1)
from concourse.dve_uop import DveOpSpec
from concourse.bass_utils import run_bass_kernel_spmd

B, D, H, HD = 4, 256, 8, 32
HPS = 4  # heads per stack (per core)
LOG2E = 1.4426950408889634
LN2 = 0.6931471805599453
QT = 512  # q-tile width
KT = 128  # k-tile width
NCH = QT // KT  # 128-q chunks per q-block
KEXP = 32  # exp2 split factor: exp2(y) = p(y/KEXP)^KEXP on the DVE path
LAG = 3  # PV issue lags scores/exp by this many events

# minimax coefficients for p(z) = 1 + z(a + z(b + z(c + d z))) ~ 2^z, |z|<=0.5
PA, PB, PC, PD = 0.693128038, 0.24023678, 0.055870371, 0.009590248

BF16 = mybir.dt.bfloat16
F32 = mybir.dt.float32
AF = mybir.ActivationFunctionType
OP = mybir.AluOpType

_prog_cache: dict = {}


def _register_dve_op(name, spec, subdim=False):
    """Register a custom DVE op at import time, self-pinning its uops sha."""
    for op in dve_ops.OPS:
        if op.name == name:
            return op
    row = dve_ops._CUSTOM_DVE_ROW_BASE + len(dve_ops.OPS)
    shas = {}
    for ver in ("v3", "v4"):
        s = DveOpSpec(name=name, opcode=row, uops=lower(spec, ver=ver),
                      rd1_en=has_src1(spec))
        shas[ver] = s.sha(ver)
    op = dve_ops.DveOp(name, spec, subdim=subdim, uops_sha=shas)
    dve_ops.OPS.append(op)
    dve_ops._SUB_OPCODE_FOR_NAME[name] = row
    dve_ops.CUSTOM_DVE_SPECS[name] = spec
    return op


def _make_exp_ops():
    z = Src0
    poly = One + z * (C0 + z * (C1 + z * (C2 + C3 * z)))
    k1 = _register_dve_op("ANT_EXP2_POLY", Spec(
        body=_spill_c3_to_src1(poly),
        reference=lambda in0, in1, s0, s1, imm2:
            1.0 + in0 * (s0 + in0 * (s1 + in0 * (
                imm2 + np.reshape(in1, (-1,) + (1,) * (in0.ndim - 1)) * in0))),
    ))
    p = Src0
    for _ in range(5):
        p = p * p
    k2m = _register_dve_op("ANT_EXP2_SQ5M", Spec(
        body=p * (Idx >= C0),
        reference=lambda in0, in1, s0, s1, imm2:
            (in0.astype(np.float32) ** 32)
            * (np.arange(in0.shape[-1], dtype=np.float32)
               >= np.reshape(s0, (-1,) + (1,) * (in0.ndim - 1))),
    ))
    p = Src0
    for _ in range(5):
        p = p * p
    k2 = _register_dve_op("ANT_EXP2_SQ5", Spec(
        body=p,
        reference=lambda in0, in1, s0, s1, imm2: in0.astype(np.float32) ** 32,
    ))
    return k1, k2m, k2


EXP2P, EXP2SQ5M, EXP2SQ5 = _make_exp_ops()


class _Busy:
    """Build-time engine busy estimates for greedy assignment (ns)."""

    def __init__(self):
        self.t = {"act": 0.0, "dve": 0.0, "pool": 0.0}

    def pick(self, costs):
        """costs: dict engine->ns; returns engine minimizing busy+cost."""
        e = min(costs, key=lambda k: self.t[k] + costs[k])
        self.t[e] += costs[e]
        return e

    def add(self, engine, ns):
        self.t[engine] += ns


def _build_program(T, causal=True):
    nc = bacc.Bacc("TRN2", target_bir_lowering=False, debug=False)

    # x layout: [128, nt, j, 512] so one DMA per 512-col t-chunk brings both
    # channel halves; wall packed as [128, 1280] (one DMA).
    NQT = T // QT
    NTT = T // KT
    xc = nc.dram_tensor("xc", [128, NQT, 2, QT], BF16, kind="ExternalInput")
    xk = nc.dram_tensor("xk", [128, NQT, 2, QT], BF16, kind="ExternalInput")
    wall = nc.dram_tensor("wall", [128, 1280], BF16, kind="ExternalInput")
    wo2 = nc.dram_tensor("wo2", [128, 512], BF16, kind="ExternalInput")
    bo_r = nc.dram_tensor("bo_r", [1, 256], BF16, kind="ExternalInput")
    pio = nc.dram_tensor("pio", [128, 1], F32, kind="ExternalInput")
    mzro = nc.dram_tensor("mzro", [128, 2 * KT], BF16, kind="ExternalInput")
    y = nc.dram_tensor("y", [T, 256], F32, kind="ExternalOutput")

    bz = _Busy()

    with tile.TileContext(nc) as tc:
        with (
            tc.tile_pool(name="xin", bufs=1) as xin,
            tc.tile_pool(name="wts", bufs=1) as wts,
            tc.tile_pool(name="proj", bufs=1) as proj,
            tc.tile_pool(name="exps", bufs=6) as exps,
            tc.tile_pool(name="ex1p", bufs=3) as ex1p,
            tc.tile_pool(name="onrm", bufs=2) as onrm,
            tc.tile_pool(name="ontp", bufs=2) as ontp,
            tc.tile_pool(name="recp", bufs=4) as recp,
            tc.tile_pool(name="yout", bufs=3) as yout,
            tc.tile_pool(name="ps_sc", bufs=3, space="PSUM") as ps_sc,
            tc.tile_pool(name="ps_pv", bufs=1, space="PSUM") as ps_pv,
        ):
            # ---- DMA order: weights, then the x chunks the first
            # projections need, then the small aux tensors, then the rest ----
            w2_sb = wts.tile([128, 2, 640], BF16, tag="w2")
            nc.sync.dma_start(w2_sb[:], wall[:].rearrange("p (j c) -> p j c",
                                                          j=2))
            w_sb = [w2_sb[:, 0, :], w2_sb[:, 1, :]]

            xc_sb = xin.tile([128, NQT, 2, QT], BF16, tag="xcs")
            xk_sb = xin.tile([128, NQT, 2, QT], BF16, tag="xks")
            nc.sync.dma_start(xc_sb[:, 0], xc[:, 0])
            nc.sync.dma_start(xk_sb[:, 0], xk[:, 0])

            pio_sb = wts.tile([128, 1], F32, tag="pio")
            mzro_sb = wts.tile([128, 2 * KT], BF16, tag="mzro")
            nc.sync.dma_start(pio_sb[:], pio[:])
            nc.sync.dma_start(mzro_sb[:], mzro[:])
            for nt in range(1, NQT):
                nc.sync.dma_start(xc_sb[:, nt], xc[:, nt])
                nc.sync.dma_start(xk_sb[:, nt], xk[:, nt])
            wo2_sb = wts.tile([128, 512], BF16, tag="wo2")
            bo_sb = wts.tile([1, 256], BF16, tag="bo")
            nc.sync.dma_start(wo2_sb[:], wo2[:])
            nc.sync.dma_start(bo_sb[:], bo_r[:])

            # warm the ACT exp table while DMAs stream in
            warm = wts.tile([128, 1], F32, tag="warm")
            nc.vector.memset(warm[:], 0.0)
            nc.scalar.activation(warm[:], warm[:], AF.Exp, scale=1.0)

            dco_sb = wts.tile([128, 1], F32, tag="dco")
            nc.vector.memset(dco_sb[:], PD)
            ones1 = wts.tile([1, 128], BF16, tag="ones1")
            nc.vector.memset(ones1[:], 1.0)

            # ---- projection targets ----
            q_c = proj.tile([128, T], BF16, tag="p_qc")
            k_c = proj.tile([128, T], BF16, tag="p_kc")
            q_k = proj.tile([128, T], BF16, tag="p_qk")
            k_k = proj.tile([128, T], BF16, tag="p_kk")
            # V with ones column: [128 t, kt, h, 33]
            v33 = proj.tile([128, NTT, HPS, 33], BF16, tag="p_v")
            nc.vector.memset(v33[:, :, :, 32:33], 1.0)
            bz.add("dve", 180.0)

            def issue_proj_group(br, nt):
                """Q and K projections for t-tile nt of one branch (packed in
                one PSUM tile), plus the V t-tiles for block nt on the c
                branch. Copies release the PSUM tile fast: q on ACT, k on
                DVE, V on Pool."""
                xs = xc_sb if br == "c" else xk_sb
                qdst, kdst = (q_c, k_c) if br == "c" else (q_k, k_k)
                qi, ki = (0, 1) if br == "c" else (2, 3)
                ps = ps_sc.tile([128, 2 * QT], F32, tag="sc")
                sl = bass.ts(nt, QT)
                for half, nm_i in ((0, qi), (1, ki)):
                    wsl = bass.ts(nm_i, 128)
                    psl = bass.ts(half, QT)
                    nc.tensor.matmul(ps[:, psl], w_sb[0][:, wsl], xs[:, nt, 0],
                                     start=True, stop=False,
                                     skip_group_check=True)
                    nc.tensor.matmul(ps[:, psl], w_sb[1][:, wsl], xs[:, nt, 1],
                                     start=False, stop=(half == 1),
                                     skip_group_check=True)
                nc.scalar.copy(qdst[:, sl], ps[:, 0:QT])
                nc.vector.tensor_copy(kdst[:, sl], ps[:, QT:2 * QT])
                bz.add("act", 612.0)
                bz.add("dve", 658.0)
                if br == "c":
                    # V: out[t, i] = sum_j x^T[j, t] * Wv.T[j, i]
                    psv = ps_sc.tile([128, 2 * QT], F32, tag="sc")
                    for i in range(4):
                        isl = bass.ts(i, KT)
                        osl = bass.ts(i, 128)
                        nc.tensor.matmul(psv[:, osl], xc_sb[:, nt, 0, isl],
                                         w_sb[0][:, 512:640],
                                         start=(i == 0), stop=False,
                                         skip_group_check=True)
                        nc.tensor.matmul(psv[:, osl], xc_sb[:, nt, 1, isl],
                                         w_sb[1][:, 512:640],
                                         start=False, stop=(i == 3),
                                         skip_group_check=True)
                    pvv = psv[:, 0:QT].rearrange("p (t c) -> p t c", t=4)
                    pvv = pvv.rearrange("p t (h d) -> p t h d", h=HPS)
                    veng = bz.pick({"act": 612.0, "dve": 658.0})
                    if veng == "act":
                        nc.scalar.copy(v33[:, 4 * nt:4 * nt + 4, :, 0:32],
                                       pvv)
                    else:
                        nc.vector.tensor_copy(
                            v33[:, 4 * nt:4 * nt + 4, :, 0:32], pvv)

            # ---- unified event stream across all (q-block, branch) ----
            segs = []
            for qt in range(NQT):
                nkt = (qt * QT + QT) // KT if causal else NTT
                for br in ("c", "k"):
                    segs.append((qt, br, nkt))
            nseg = len(segs)

            pvq = []      # deferred PV work crossing segment boundaries
            post = {}     # seg-local step -> list of fns (boundary work)
            tail_evs = 4  # bias the very last events to the fast engine

            def issue_pv(ctx, ex, kt, hp, di):
                pv, started, nkt, _ = ctx
                clo = di if di > 0 else 0
                for hl in range(2):
                    h = 2 * hp + hl
                    for c in range(clo, NCH):
                        st = not started[hp]
                        started[hp] = True
                        stop = (hl == 1 and c == NCH - 1 and kt == nkt - 1)
                        nc.tensor.matmul(
                            pv[hp][:, c, hl, :],
                            ex[:, hl, KT * c:KT * (c + 1)],
                            v33[:, kt, h, :],
                            start=st, stop=stop,
                            skip_group_check=True,
                        )

            def make_norm(ctx):
                pv, _, _, (qt, br) = ctx
                onT = onrm.tile([128, NCH, HPS, 32], BF16, tag=f"on{br}")
                on = ontp.tile([128, QT], BF16, tag=f"ot{br}")

                def norm():
                    for hp in range(2):
                        rec = recp.tile([128, NCH, 2, 1], F32, tag="rec")
                        nc.vector.reciprocal_approx_fast(
                            rec[:].rearrange("p a b c -> p (a b c)"),
                            pv[hp][:, :, :, 32:33].rearrange(
                                "p a b c -> p (a b c)"))
                        nc.vector.tensor_tensor(
                            onT[:, :, 2 * hp:2 * hp + 2, :],
                            pv[hp][:, :, :, 0:32],
                            rec[:].broadcast_to([128, NCH, 2, 32]),
                            OP.mult)
                        bz.add("dve", 700.0)

                def trans():
                    for c in range(NCH):
                        nc.sync.dma_start_transpose(
                            on[:, KT * c:KT * (c + 1)],
                            onT[:, c, :, :])
                return onT, on, norm, trans

            def make_outproj(qt, on_c, on_k):
                q0 = qt * QT
                yps = {}

                def proj_m(m):
                    ysl = slice(256 * (m % 2), 256 * (m % 2) + 256)
                    if m % 2 == 0:
                        yps[m // 2] = ps_pv.tile(
                            [128, 512], F32, tag=f"pv{m // 2}",
                            name=f"yp{qt}_{m // 2}")
                    yp = yps[m // 2]
                    nc.tensor.matmul(
                        yp[:, ysl], ones1[:], bo_sb[:],
                        start=(m % 2 == 0), stop=False,
                        skip_group_check=True)
                    nc.tensor.matmul(
                        yp[:, ysl],
                        on_c[:, bass.ts(m, 128)], wo2_sb[:, 0:256],
                        start=False, stop=False,
                        skip_group_check=True)
                    nc.tensor.matmul(
                        yp[:, ysl],
                        on_k[:, bass.ts(m, 128)], wo2_sb[:, 256:512],
                        start=False, stop=(m % 2 == 1),
                        skip_group_check=True)
                    ysb = yout.tile([128, 256], F32, tag="y")
                    ceng = bz.pick({"act": 398.0, "dve": 390.0})
                    if ceng == "act":
                        nc.scalar.copy(ysb[:], yp[:, ysl])
                    else:
                        nc.vector.tensor_copy(ysb[:], yp[:, ysl])
                    nc.sync.dma_start(
                        y[q0 + m * 128:q0 + (m + 1) * 128, :], ysb[:])
                return proj_m

            ev_total = sum(2 * nkt for _, _, nkt in segs)
            ev_done = 0
            prev_ctx = None   # (pv, started, nkt, (qt, br)) of previous seg
            prev_norm = None  # (onT, on, norm, trans) of previous seg
            on_done = {}      # (qt, br) -> on tile

            for si, (qt, br, nkt) in enumerate(segs):
                q0 = qt * QT
                qsb, ksb = (q_c, k_c) if br == "c" else (q_k, k_k)
                issue_proj_group(br, qt)

                pv0 = ps_pv.tile([128, NCH, 2, 33], F32, tag="pv0")
                pv1 = ps_pv.tile([128, NCH, 2, 33], F32, tag="pv1")
                ctx = ([pv0, pv1], [False, False], nkt, (qt, br))

                # schedule previous segment's boundary work into this stream
                post.clear()
                if prev_ctx is not None:
                    pn = prev_norm

                    def mk(fn):
                        return fn
                    post.setdefault(LAG, []).append(pn[2])       # norm
                    post.setdefault(LAG + 1, []).append(pn[3])   # transposes
                    pqt, pbr = prev_ctx[3]
                    on_done[(pqt, pbr)] = pn[1]
                    if pbr == "k":
                        pm = make_outproj(pqt, on_done[(pqt, "c")],
                                          on_done[(pqt, "k")])
                        for m in range(4):
                            post.setdefault(LAG + 2 + m, []).append(
                                mk(lambda m=m, pm=pm: pm(m)))

                ndiag = 4 if causal else 0
                step = 0
                for kt in range(nkt):
                    k0 = kt * KT
                    di = kt - (nkt - ndiag) if causal else -1
                    qlo = 128 * di if di > 0 else 0
                    w = 2 * (QT - qlo)
                    for hp in range(2):
                        sp = ps_sc.tile([128, 2 * QT], F32, tag="sc")
                        for hl in range(2):
                            h = 2 * hp + hl
                            nc.tensor.matmul(
                                sp[:, QT * hl + qlo: QT * (hl + 1)],
                                ksb[32 * h:32 * h + 32, k0:k0 + KT],
                                qsb[32 * h:32 * h + 32, q0 + qlo:q0 + QT],
                                start=True, stop=True,
                                tile_position=(32 * h, 0),
                                skip_group_check=True,
                            )
                        ex = exps.tile([128, 2, QT], BF16, tag="ex")
                        spv = sp[:].rearrange("p (l q) -> p l q", l=2)
                        c_act = 0.833 * w + 185.0
                        c_dve = 2.083 * w + 245.0
                        if ev_total - ev_done <= tail_evs:
                            # keep the global tail off the slow engine
                            c_dve *= 2.0
                        ev_done += 1
                        costs = {"act": c_act + (60.0 if di >= 0 else 0.0),
                                 "dve": c_dve}
                        eng = bz.pick(costs)
                        if eng == "act":
                            nc.scalar.activation(ex[:, :, qlo:],
                                                 spv[:, :, qlo:],
                                                 AF.Exp, scale=KEXP * LN2)
                            if di >= 0:
                                exv = ex[:, :, qlo:qlo + KT]
                                mzv = mzro_sb[:].rearrange(
                                    "p (l q) -> p l q", l=2)
                                meng = bz.pick({"pool": 603.0,
                                                "dve": 200.0})
                                if meng == "pool":
                                    nc.gpsimd.tensor_tensor(
                                        exv, exv, mzv, OP.mult)
                                else:
                                    nc.vector.tensor_tensor(
                                        exv, exv, mzv, OP.mult)
                        else:
                            e1 = ex1p.tile([128, 2, QT], F32, tag="e1")
                            nc.vector._custom_dve(
                                EXP2P, out=e1[:, :, qlo:],
                                in0=spv[:, :, qlo:],
                                in1=dco_sb[:, 0:1], s0=PA, s1=PB, imm2=PC)
                            if di >= 0:
                                for hl in range(2):
                                    nc.vector._custom_dve(
                                        EXP2SQ5M,
                                        out=ex[:, hl, qlo:],
                                        in0=e1[:, hl, qlo:],
                                        s0=pio_sb[:, 0:1])
                            else:
                                nc.vector._custom_dve(
                                    EXP2SQ5, out=ex[:], in0=e1[:])
                        # boundary work first: norm(prev) must be issued
                        # before the first PV write of this segment's pv
                        # banks (WAR ordering on the shared pv tags)
                        for fn in post.pop(step, ()):
                            fn()
                        # PV + denominator deferred by LAG events
                        pvq.append((ctx, ex, kt, hp, di))
                        if len(pvq) > LAG:
                            issue_pv(*pvq.pop(0))
                        step += 1

                # boundary work that did not fit in a short segment
                for s in sorted(post):
                    for fn in post[s]:
                        fn()
                post.clear()

                prev_ctx = ctx
                prev_norm = make_norm(ctx)

            # drain: last segment's PV, norm, transposes, out-proj
            while pvq:
                issue_pv(*pvq.pop(0))
            prev_norm[2]()
            prev_norm[3]()
            pqt, pbr = prev_ctx[3]
            on_done[(pqt, pbr)] = prev_norm[1]
            pm = make_outproj(pqt, on_done[(pqt, "c")], on_done[(pqt, "k")])
            for m in range(4):
                pm(m)

    nc.compile()
    return nc


def _bf(x):
    return np.ascontiguousarray(np.asarray(x, np.float32)).astype(ml_dtypes.bfloat16)


def _host_prep(inputs, T):
    content = np.asarray(inputs["content"], np.float32)
    category = np.asarray(inputs["category"], np.float32)
    Wqc = np.asarray(inputs["Wqc"], np.float32)
    Wkc = np.asarray(inputs["Wkc"], np.float32)
    Wv = np.asarray(inputs["Wv"], np.float32)
    Wqk = np.asarray(inputs["Wqk"], np.float32)
    Wkk = np.asarray(inputs["Wkk"], np.float32)
    Wo = np.asarray(inputs["Wo"], np.float32)
    bo = np.asarray(inputs["bo"], np.float32)
    alpha = 1.0 / (1.0 + np.exp(-float(np.asarray(inputs["alpha_logit"]))))
    nb = content.shape[0]

    scale_q = (HD ** -0.5) * LOG2E / KEXP

    pio = np.arange(128, dtype=np.float32)[:, None]
    p_idx = np.arange(128)[:, None]
    qcol = np.arange(KT)[None, :]
    mzro = np.tile((qcol >= p_idx).astype(np.float32), (1, 2))
    mzro = _bf(mzro)

    in_maps = []
    for core in range(2 * nb):
        b, s = core // 2, core % 2
        # wall[j] = [Wqc | Wkc | Wqk | Wkk | Wv].T chunks for x-chan half j
        wj = []
        for j in range(2):
            cols = []
            for W, sc in [(Wqc, scale_q), (Wkc, 1.0), (Wqk, scale_q),
                          (Wkk, 1.0), (Wv, 1.0)]:
                cols.append((W.T * sc)[128 * j:128 * (j + 1),
                                       128 * s:128 * (s + 1)])
            wj.append(np.concatenate(cols, axis=1))
        wall = _bf(np.concatenate(wj, axis=1))
        wo2 = _bf(np.concatenate(
            [Wo.T[128 * s:128 * (s + 1), :] * (1.0 - alpha),
             Wo.T[128 * s:128 * (s + 1), :] * alpha], axis=1))

        def _xlay(x):
            # [D, T] -> [128 p, nt, j, 512]: p = row within channel half j
            a = x.T.reshape(2, 128, T).transpose(1, 2, 0)  # [p, t, j]
            nqt = T // QT
            return _bf(a.reshape(128, nqt, QT, 2).transpose(0, 1, 3, 2))

        m = {
            "xc": _xlay(content[b]),
            "xk": _xlay(category[b]),
            "wall": wall,
            "wo2": wo2,
            "bo_r": (_bf(bo[None, :]) if s == 0
                     else np.zeros((1, 256), ml_dtypes.bfloat16)),
            "pio": pio,
            "mzro": mzro,
        }
        in_maps.append(m)
    return in_maps


def _check_mask(mask, T):
    exp = np.triu(np.ones((T, T), dtype=bool), k=1)
    return np.array_equal(np.asarray(mask), exp)


def run(inputs, T=2048, cores=None, causal=True, **run_kwargs):
    """Build/compile (cached), run on hardware, return BassKernelResults."""
    key = (T, causal)
    if key not in _prog_cache:
        _prog_cache[key] = _build_program(T, causal=causal)
    nc = _prog_cache[key]
    in_maps = _host_prep(inputs, T)
    if cores is None:
        cores = list(range(len(in_maps)))
    res = run_bass_kernel_spmd(nc, [in_maps[c] for c in cores],
                               core_ids=list(range(len(cores))), **run_kwargs)
    return res


def kernel(**inputs):
    T = 2048
    mask = np.asarray(inputs["causal_mask"])
    if _check_mask(mask, T):
        causal = True
    elif not mask.any():
        causal = False
    else:
        raise NotImplementedError("kernel supports causal or empty masks only")
    res = run(inputs, T=T, causal=causal)
    nb = np.asarray(inputs["content"]).shape[0]
    out = np.empty((nb, T, D), np.float32)
    for b in range(nb):
        out[b] = res.results[2 * b]["y"] + res.results[2 * b + 1]["y"]
    return out


# revision 47
# speedup vs baseline: 1.3729x; 1.1039x over previous
"""DIF (dual-softmax) attention layer on 8 Trainium2 NeuronCores.

Sharding: core = (batch b, head-stack s), b in 0..3, s in 0..1.
Each core computes, for its batch and its 4 heads, the full dual-softmax
attention over all T rows, producing a partial output projection (sum over
its 4 heads; bias folded into stack 0). Host sums the two stack partials.

On-chip layout, "transposed-PV" design with a unified software-pipelined
event stream. The cost model charges a matmul by its output free size only,
so PV runs with q on the OUTPUT PARTITIONS:
  - Q^T, K^T per branch: [128 (4h x 32d), T] bf16; attention scale * log2(e)
    folded into the Q projection weights (exp runs in base 2).
  - events: one per (q-block, branch, k-tile, head-pair). Per event, 2
    row-packed score matmuls (tile_position=(32h,0)) into one [128, 1024]
    PSUM tile, then exp on ScalarE (AF.Exp) OR the 2-pass custom-DVE exp
    (poly + ^32), chosen by a greedy busy-balance. Diagonal tiles restrict
    to the valid q range; the 128-col triangle mask is in-op (DVE Idx) or an
    external multiply (Pool/DVE) for the ACT path.
  - PV+denominator: deferred by LAG events (so the in-order PE queue never
    blocks upcoming score matmuls behind a PV waiting on a slow exp). Per
    (event, hl, 128-q-chunk) ONE matmul
      out[128 q, 33] += ex[:, hl, chunk].T @ v33[:, kt, h, :]
    where v33 carries V^T and a ones-column (col 32) so the softmax
    denominator accumulates as column 32 of the same PSUM tile.
  - PSUM tetris (8 banks): score/proj/out-proj tiles all rotate through one
    3-deep [128,1024] f32 pool (6 banks); pv0/pv1 keep one bank each.
  - boundary work (normalize, DMA transposes back to [128 (h,d), q], output
    projection) is injected at fixed offsets into the NEXT segment's event
    stream so it overlaps and never head-blocks an engine queue.
  - QKV projection groups are issued per segment on demand so attention
    starts as soon as the first x chunks land; PSUM->SBUF copies run on
    Pool; y stores go out on the gpsimd SWDGE queue.
"""

import numpy as np
import ml_dtypes

import concourse.bass as bass
import concourse.tile as tile
from concourse import bacc, mybir, dve_ops
from concourse.dve_spec import (Spec, Src0, C0, C1, C2, C3, One, Idx,
                                lower, _spill_c3_to_src1, _has_src1 as has_src1)
from concourse.dve_uop import DveOpSpec
from concourse.bass_utils import run_bass_kernel_spmd

B, D, H, HD = 4, 256, 8, 32
HPS = 4  # heads per stack (per core)
LOG2E = 1.4426950408889634
LN2 = 0.6931471805599453
QT = 512  # q-tile width
KT = 128  # k-tile width
NCH = QT // KT  # 128-q chunks per q-block
KEXP = 32  # exp2 split factor: exp2(y) = p(y/KEXP)^KEXP on the DVE path
NORM_STEP = 4  # seg-local step at which the previous segment's norm issues
# PV issue lag (events) by exp engine: slower exps get a deeper lag so the
# in-order PE queue never idles on them; masked (diagonal) events carry the
# extra mask-op latency
LAG_ACT, LAG_DVE, LAG_MASK = 3, 4, 6

# minimax coefficients for p(z) = 1 + z(a + z(b + z(c + d z))) ~ 2^z, |z|<=0.5
PA, PB, PC, PD = 0.693128038, 0.24023678, 0.055870371, 0.009590248

BF16 = mybir.dt.bfloat16
F16 = mybir.dt.float16
F32 = mybir.dt.float32
AF = mybir.ActivationFunctionType
OP = mybir.AluOpType

_prog_cache: dict = {}


def _register_dve_op(name, spec, subdim=False):
    """Register a custom DVE op at import time, self-pinning its uops sha."""
    for op in dve_ops.OPS:
        if op.name == name:
            return op
    row = dve_ops._CUSTOM_DVE_ROW_BASE + len(dve_ops.OPS)
    shas = {}
    for ver in ("v3", "v4"):
        s = DveOpSpec(name=name, opcode=row, uops=lower(spec, ver=ver),
                      rd1_en=has_src1(spec))
        shas[ver] = s.sha(ver)
    op = dve_ops.DveOp(name, spec, subdim=subdim, uops_sha=shas)
    dve_ops.OPS.append(op)
    dve_ops._SUB_OPCODE_FOR_NAME[name] = row
    dve_ops.CUSTOM_DVE_SPECS[name] = spec
    return op


def _make_exp_ops():
    # Single-pass 2^(16 z) ~ (q0 + z(qa + qb z))^16: quadratic relative-
    # minimax fit on |z| <= 0.362 (score range +-8 at KEXP=32) plus four
    # in-op squarings -- exactly 8 ALU stages. The final ^2 (to reach
    # KEXP=32) runs as a separate fp16 TensorTensor in the DVE 2x mode.
    z = Src0
    p = C2 + z * (C0 + C1 * z)
    for _ in range(4):
        p = p * p
    k1 = _register_dve_op("ANT_EXP2_Q4", Spec(
        body=p,
        reference=lambda in0, in1, s0, s1, imm2:
            (imm2 + in0 * (s0 + s1 * in0)) ** 16,
    ))
    return k1


EXP2Q4 = _make_exp_ops()
QC0, QC1, QC2 = 1.00014658, 0.69963674, 0.23982307


class _Busy:
    """Build-time engine busy estimates for greedy assignment (ns)."""

    def __init__(self):
        self.t = {"act": 0.0, "dve": 0.0, "pool": 0.0}

    def pick(self, costs):
        """costs: dict engine->ns; returns engine minimizing busy+cost."""
        e = min(costs, key=lambda k: self.t[k] + costs[k])
        self.t[e] += costs[e]
        return e

    def add(self, engine, ns):
        self.t[engine] += ns


def _build_program(T, causal=True):
    nc = bacc.Bacc("TRN2", target_bir_lowering=False, debug=False)

    # x layout: [128, nt, j, 512] so one DMA per 512-col t-chunk brings both
    # channel halves; wall packed as [128, 1280] (one DMA).
    NQT = T // QT
    NTT = T // KT
    xc = nc.dram_tensor("xc", [128, NQT, 2, QT], BF16, kind="ExternalInput")
    xk = nc.dram_tensor("xk", [128, NQT, 2, QT], BF16, kind="ExternalInput")
    wqkc = nc.dram_tensor("wqkc", [128, 512], BF16, kind="ExternalInput")
    wv = nc.dram_tensor("wv", [128, 256], BF16, kind="ExternalInput")
    wqkk = nc.dram_tensor("wqkk", [128, 512], BF16, kind="ExternalInput")
    wo2 = nc.dram_tensor("wo2", [128, 512], BF16, kind="ExternalInput")
    bo_r = nc.dram_tensor("bo_r", [1, 256], BF16, kind="ExternalInput")
    mzro = nc.dram_tensor("mzro", [128, 2 * KT], F16, kind="ExternalInput")
    id128 = nc.dram_tensor("id128", [128, 128], BF16, kind="ExternalInput")
    y = nc.dram_tensor("y", [T, 256], F32, kind="ExternalOutput")

    bz = _Busy()

    with tile.TileContext(nc) as tc:
        with (
            tc.tile_pool(name="xin", bufs=1) as xin,
            tc.tile_pool(name="wts", bufs=1) as wts,
            tc.tile_pool(name="proj", bufs=1) as proj,
            tc.tile_pool(name="exps", bufs=6) as exps,
            tc.tile_pool(name="ex1p", bufs=3) as ex1p,
            tc.tile_pool(name="onrm", bufs=2) as onrm,
            tc.tile_pool(name="ontp", bufs=2) as ontp,
            tc.tile_pool(name="recp", bufs=4) as recp,
            tc.tile_pool(name="yout", bufs=3) as yout,
            tc.tile_pool(name="ps_sc", bufs=3, space="PSUM") as ps_sc,
            tc.tile_pool(name="ps_pv", bufs=1, space="PSUM") as ps_pv,
        ):
            # ---- DMA order tracks first use: [Wqc|Wkc], x_c chunk0, Wv,
            # x_k chunk0, [Wqk|Wkk], aux, remaining x, output weights ----
            wqkc_sb = wts.tile([128, 2, 256], BF16, tag="wqkc")
            nc.sync.dma_start(wqkc_sb[:],
                              wqkc[:].rearrange("p (j c) -> p j c", j=2))
            xc_sb = xin.tile([128, NQT, 2, QT], BF16, tag="xcs")
            xk_sb = xin.tile([128, NQT, 2, QT], BF16, tag="xks")
            nc.sync.dma_start(xc_sb[:, 0], xc[:, 0])
            wv_sb = wts.tile([128, 2, 128], BF16, tag="wv")
            nc.sync.dma_start(wv_sb[:],
                              wv[:].rearrange("p (j c) -> p j c", j=2))
            nc.sync.dma_start(xk_sb[:, 0], xk[:, 0])
            wqkk_sb = wts.tile([128, 2, 256], BF16, tag="wqkk")
            nc.sync.dma_start(wqkk_sb[:],
                              wqkk[:].rearrange("p (j c) -> p j c", j=2))

            mzro_sb = wts.tile([128, 2 * KT], F16, tag="mzro")
            nc.sync.dma_start(mzro_sb[:], mzro[:])
            for nt in range(1, NQT):
                nc.sync.dma_start(xc_sb[:, nt], xc[:, nt])
                nc.sync.dma_start(xk_sb[:, nt], xk[:, nt])
            wo2_sb = wts.tile([128, 512], BF16, tag="wo2")
            bo_sb = wts.tile([1, 256], BF16, tag="bo")
            id_sb = wts.tile([128, 128], BF16, tag="id128")
            nc.sync.dma_start(wo2_sb[:], wo2[:])
            nc.sync.dma_start(bo_sb[:], bo_r[:])
            nc.sync.dma_start(id_sb[:], id128[:])

            # warm the ACT exp table while DMAs stream in
            warm = wts.tile([128, 1], F32, tag="warm")
            nc.vector.memset(warm[:], 0.0)
            nc.scalar.activation(warm[:], warm[:], AF.Exp, scale=1.0)

            ones1 = wts.tile([1, 128], BF16, tag="ones1")
            nc.vector.memset(ones1[:], 1.0)

            # ---- projection targets (q,k packed per branch: one copy) ----
            qk_c = proj.tile([128, 2, T], BF16, tag="p_qkc")
            qk_k = proj.tile([128, 2, T], BF16, tag="p_qkk")
            q_c, k_c = qk_c[:, 0, :], qk_c[:, 1, :]
            q_k, k_k = qk_k[:, 0, :], qk_k[:, 1, :]
            # V with ones column: [128 t, kt, h, 33]
            v33 = proj.tile([128, NTT, HPS, 33], F16, tag="p_v")
            nc.vector.memset(v33[:, :, :, 32:33], 1.0)
            bz.add("dve", 180.0)

            def issue_proj_group(br, nt):
                """Q and K projections for t-tile nt of one branch (packed in
                one PSUM tile), plus the V t-tiles for block nt on the c
                branch. Copies release the PSUM tile fast: q on ACT, k on
                DVE, V on Pool."""
                xs = xc_sb if br == "c" else xk_sb
                qkdst = qk_c if br == "c" else qk_k
                wqk = wqkc_sb if br == "c" else wqkk_sb
                ps = ps_sc.tile([128, 2 * QT], F32, tag="sc")
                sl = bass.ts(nt, QT)
                for half in range(2):
                    wsl = bass.ts(half, 128)
                    psl = bass.ts(half, QT)
                    nc.tensor.matmul(ps[:, psl], wqk[:, 0, wsl], xs[:, nt, 0],
                                     start=True, stop=False,
                                     skip_group_check=True)
                    nc.tensor.matmul(ps[:, psl], wqk[:, 1, wsl], xs[:, nt, 1],
                                     start=False, stop=(half == 1),
                                     skip_group_check=True)
                psqk = ps[:].rearrange("p (h q) -> p h q", h=2)
                ceng = bz.pick({"act": 1038.0, "dve": 1311.0})
                if ceng == "act":
                    nc.scalar.copy(qkdst[:, :, sl], psqk)
                else:
                    nc.vector.tensor_copy(qkdst[:, :, sl], psqk)
                if br == "c":
                    # V: out[t, i] = sum_j x^T[j, t] * Wv.T[j, i]
                    psv = ps_sc.tile([128, 2 * QT], F32, tag="sc")
                    for i in range(4):
                        isl = bass.ts(i, KT)
                        osl = bass.ts(i, 128)
                        nc.tensor.matmul(psv[:, osl], xc_sb[:, nt, 0, isl],
                                         wv_sb[:, 0, :],
                                         start=(i == 0), stop=False,
                                         skip_group_check=True)
                        nc.tensor.matmul(psv[:, osl], xc_sb[:, nt, 1, isl],
                                         wv_sb[:, 1, :],
                                         start=False, stop=(i == 3),
                                         skip_group_check=True)
                    pvv = psv[:, 0:QT].rearrange("p (t c) -> p t c", t=4)
                    pvv = pvv.rearrange("p t (h d) -> p t h d", h=HPS)
                    veng = bz.pick({"act": 612.0, "dve": 658.0})
                    if veng == "act":
                        nc.scalar.copy(v33[:, 4 * nt:4 * nt + 4, :, 0:32],
                                       pvv)
                    else:
                        nc.vector.tensor_copy(
                            v33[:, 4 * nt:4 * nt + 4, :, 0:32], pvv)

            # ---- unified event stream across all (q-block, branch) ----
            segs = []
            for qt in range(NQT):
                nkt = (qt * QT + QT) // KT if causal else NTT
                for br in ("c", "k"):
                    segs.append((qt, br, nkt))
            nseg = len(segs)

            pvq = []      # deferred PV work crossing segment boundaries
            post = {}     # seg-local step -> list of fns (boundary work)
            tail_evs = 4  # bias the very last events to the fast engine

            def issue_pv(ctx, ex, kt, hp, di):
                pv, started, nkt, _ = ctx
                clo = di if di > 0 else 0
                for hl in range(2):
                    h = 2 * hp + hl
                    for c in range(clo, NCH):
                        st = not started[hp]
                        started[hp] = True
                        stop = (hl == 1 and c == NCH - 1 and kt == nkt - 1)
                        nc.tensor.matmul(
                            pv[hp][:, c, hl, :],
                            ex[:, hl, KT * c:KT * (c + 1)],
                            v33[:, kt, h, :],
                            start=st, stop=stop,
                            skip_group_check=True,
                        )

            def make_norm(ctx):
                pv, _, _, (qt, br) = ctx
                onT = onrm.tile([128, NCH, HPS, 32], BF16, tag=f"on{br}")
                on = ontp.tile([128, QT], BF16, tag=f"ot{br}")

                def norm():
                    for hp in range(2):
                        rec = recp.tile([128, NCH, 2, 1], F32, tag="rec")
                        nc.vector.reciprocal_approx_fast(
                            rec[:].rearrange("p a b c -> p (a b c)"),
                            pv[hp][:, :, :, 32:33].rearrange(
                                "p a b c -> p (a b c)"))
                        nc.vector.tensor_tensor(
                            onT[:, :, 2 * hp:2 * hp + 2, :],
                            pv[hp][:, :, :, 0:32],
                            rec[:].broadcast_to([128, NCH, 2, 32]),
                            OP.mult)
                        bz.add("dve", 700.0)

                def trans():
                    for c in range(NCH):
                        nc.sync.dma_start_transpose(
                            on[:, KT * c:KT * (c + 1)],
                            onT[:, c, :, :])
                return onT, on, norm, trans

            def make_outproj(qt, on_c, on_k):
                q0 = qt * QT
                yps = {}

                def proj_m(m):
                    ysl = slice(256 * (m % 2), 256 * (m % 2) + 256)
                    if m % 2 == 0:
                        yps[m // 2] = ps_pv.tile(
                            [128, 512], F32, tag=f"pv{m // 2}",
                            name=f"yp{qt}_{m // 2}")
                    yp = yps[m // 2]
                    nc.tensor.matmul(
                        yp[:, ysl], ones1[:], bo_sb[:],
                        start=(m % 2 == 0), stop=False,
                        skip_group_check=True)
                    nc.tensor.matmul(
                        yp[:, ysl],
                        on_c[:, bass.ts(m, 128)], wo2_sb[:, 0:256],
                        start=False, stop=False,
                        skip_group_check=True)
                    nc.tensor.matmul(
                        yp[:, ysl],
                        on_k[:, bass.ts(m, 128)], wo2_sb[:, 256:512],
                        start=False, stop=(m % 2 == 1),
                        skip_group_check=True)
                    if m % 2 == 0:
                        return lambda: None
                    # one copy for the yp pair, two slice DMAs
                    ysb = yout.tile([128, 512], F32, tag="y")
                    ceng = bz.pick({"act": 612.0, "dve": 778.0})
                    if ceng == "act":
                        nc.scalar.copy(ysb[:], yp[:])
                    else:
                        nc.vector.tensor_copy(ysb[:], yp[:])

                    def dma():
                        for mm in (m - 1, m):
                            nc.sync.dma_start(
                                y[q0 + mm * 128:q0 + (mm + 1) * 128, :],
                                ysb[:, bass.ts(mm % 2, 256)])
                    return dma
                return proj_m

            ev_total = sum(2 * nkt for _, _, nkt in segs)
            ev_done = 0
            prev_ctx = None   # (pv, started, nkt, (qt, br)) of previous seg
            prev_norm = None  # (onT, on, norm, trans) of previous seg
            on_done = {}      # (qt, br) -> on tile
            gstep = 0         # global event counter

            for si, (qt, br, nkt) in enumerate(segs):
                q0 = qt * QT
                qsb, ksb = (q_c, k_c) if br == "c" else (q_k, k_k)
                if si < 2:
                    issue_proj_group(br, qt)

                pv0 = ps_pv.tile([128, NCH, 2, 33], F32, tag="pv0")
                pv1 = ps_pv.tile([128, NCH, 2, 33], F32, tag="pv1")
                ctx = ([pv0, pv1], [False, False], nkt, (qt, br))

                # schedule previous segment's boundary work into this stream
                post.clear()
                if si + 2 < nseg:
                    nqt2, nbr2, _ = segs[si + 2]
                    post.setdefault(7, []).append(
                        lambda b=nbr2, n=nqt2: issue_proj_group(b, n))
                if prev_ctx is not None:
                    pn = prev_norm
                    post.setdefault(NORM_STEP, []).append(pn[2])      # norm
                    post.setdefault(NORM_STEP + 1, []).append(pn[3])  # tr
                    pqt, pbr = prev_ctx[3]
                    on_done[(pqt, pbr)] = pn[1]
                    if pbr == "k":
                        pm = make_outproj(pqt, on_done[(pqt, "c")],
                                          on_done[(pqt, "k")])
                        dmas = []

                        def mkm(m):
                            def f():
                                dmas.append(pm(m))
                            return f

                        def mkd(m):
                            def f():
                                dmas[m]()
                            return f
                        for m in range(4):
                            post.setdefault(NORM_STEP + 2 + m, []).append(
                                mkm(m))
                            # y DMA holds SP.SEQ while waiting on its copy;
                            # issue it only once the copy has surely drained
                            post.setdefault(NORM_STEP + 7 + m, []).append(
                                mkd(m))

                ndiag = 4 if causal else 0
                step = 0
                nev = 2 * nkt
                for kt in range(nkt):
                    k0 = kt * KT
                    di = kt - (nkt - ndiag) if causal else -1
                    qlo = 128 * di if di > 0 else 0
                    w = 2 * (QT - qlo)
                    for hp in range(2):
                        sp = ps_sc.tile([128, 2 * QT], F32, tag="sc")
                        for hl in range(2):
                            h = 2 * hp + hl
                            nc.tensor.matmul(
                                sp[:, QT * hl + qlo: QT * (hl + 1)],
                                ksb[32 * h:32 * h + 32, k0:k0 + KT],
                                qsb[32 * h:32 * h + 32, q0 + qlo:q0 + QT],
                                start=True, stop=True,
                                tile_position=(32 * h, 0),
                                skip_group_check=True,
                            )
                        ex = exps.tile([128, 2, QT], F16, tag="ex")
                        spv = sp[:].rearrange("p (l q) -> p l q", l=2)
                        c_act = 0.833 * w + 185.0
                        c_dve = 1.56 * w + 300.0
                        if ev_total - ev_done <= tail_evs:
                            # keep the global tail off the slow engine
                            c_dve *= 2.0
                        ev_done += 1
                        costs = {"act": c_act + (60.0 if di >= 0 else 0.0),
                                 "dve": c_dve}
                        eng = bz.pick(costs)
                        if eng == "act":
                            nc.scalar.activation(ex[:, :, qlo:],
                                                 spv[:, :, qlo:],
                                                 AF.Exp, scale=KEXP * LN2)
                        else:
                            e16 = ex1p.tile([128, 2, QT], F16, tag="e1")
                            nc.vector._custom_dve(
                                EXP2Q4, out=e16[:, :, qlo:],
                                in0=spv[:, :, qlo:],
                                s0=QC1, s1=QC2, imm2=QC0)
                            nc.vector.tensor_tensor(
                                ex[:, :, qlo:], e16[:, :, qlo:],
                                e16[:, :, qlo:], OP.mult)
                        if di >= 0:
                            exv = ex[:, :, qlo:qlo + KT]
                            mzv = mzro_sb[:].rearrange(
                                "p (l q) -> p l q", l=2)
                            if eng == "dve":
                                # runs right behind the square in the DVE
                                # queue: near-zero added latency
                                meng = "dve"
                                bz.add("dve", 200.0)
                            else:
                                meng = bz.pick({"pool": 603.0, "dve": 200.0})
                            if meng == "pool":
                                nc.gpsimd.tensor_tensor(
                                    exv, exv, mzv, OP.mult)
                            else:
                                nc.vector.tensor_tensor(
                                    exv, exv, mzv, OP.mult)
                        # boundary work first: norm(prev) must be issued
                        # before the first PV write of this segment's pv
                        # banks (WAR ordering on the shared pv tags)
                        for fn in post.pop(step, ()):
                            fn()
                        # PV + denominator deferred; lag adapts to the exp
                        # engine's latency. Segment-edge clamps keep the
                        # norm/PV/WAR ordering legal: the first events of a
                        # segment must not pop before norm(prev) at
                        # NORM_STEP, the last must pop before it.
                        lag = (LAG_MASK if di >= 0
                               else LAG_DVE if eng == "dve" else LAG_ACT)
                        lag = min(lag, (nev - 1 - step) + NORM_STEP)
                        lag = max(lag, NORM_STEP + 1 - step)
                        pvq.append((gstep + lag, ctx, ex, kt, hp, di))
                        gstep += 1
                        while pvq and pvq[0][0] <= gstep:
                            issue_pv(*pvq.pop(0)[1:])
                        step += 1

                # boundary work that did not fit in a short segment
                for s in sorted(post):
                    for fn in post[s]:
                        fn()
                post.clear()

                prev_ctx = ctx
                prev_norm = make_norm(ctx)

            # drain: last segment's PV, norm, then PE-based transposes (the
            # serial DMA-transpose chain would sit on the tail critical
            # path; PSUM is free here) and the final out-proj
            while pvq:
                issue_pv(*pvq.pop(0)[1:])
            prev_norm[2]()
            onT_l, on_l = prev_norm[0], prev_norm[1]
            ptr = ps_sc.tile([128, NCH, 128], BF16, tag="sc", name="drain_tr")
            for c in range(NCH):
                nc.tensor.transpose(
                    ptr[:, c, :],
                    onT_l[:, c, :, :].rearrange("p h d -> p (h d)"),
                    id_sb[:])
                if c % 2 == 0:
                    nc.scalar.copy(on_l[:, KT * c:KT * (c + 1)], ptr[:, c, :])
                else:
                    nc.vector.tensor_copy(on_l[:, KT * c:KT * (c + 1)],
                                          ptr[:, c, :])
            pqt, pbr = prev_ctx[3]
            on_done[(pqt, pbr)] = on_l
            pm = make_outproj(pqt, on_done[(pqt, "c")], on_done[(pqt, "k")])
            for m in range(4):
                pm(m)()

    nc.compile()
    return nc


def _bf(x):
    return np.ascontiguousarray(np.asarray(x, np.float32)).astype(ml_dtypes.bfloat16)


def _host_prep(inputs, T):
    content = np.asarray(inputs["content"], np.float32)
    category = np.asarray(inputs["category"], np.float32)
    Wqc = np.asarray(inputs["Wqc"], np.float32)
    Wkc = np.asarray(inputs["Wkc"], np.float32)
    Wv = np.asarray(inputs["Wv"], np.float32)
    Wqk = np.asarray(inputs["Wqk"], np.float32)
    Wkk = np.asarray(inputs["Wkk"], np.float32)
    Wo = np.asarray(inputs["Wo"], np.float32)
    bo = np.asarray(inputs["bo"], np.float32)
    alpha = 1.0 / (1.0 + np.exp(-float(np.asarray(inputs["alpha_logit"]))))
    nb = content.shape[0]

    scale_q = (HD ** -0.5) * LOG2E / KEXP

    p_idx = np.arange(128)[:, None]
    qcol = np.arange(KT)[None, :]
    mzro = np.tile((qcol >= p_idx).astype(np.float32), (1, 2))
    mzro = np.ascontiguousarray(mzro).astype(np.float16)

    in_maps = []
    for core in range(2 * nb):
        b, s = core // 2, core % 2

        def wcols(ws):
            # j-major: [j0: W0|W1|.., j1: ...] chunks for x-chan half j
            return _bf(np.concatenate(
                [np.concatenate(
                    [(W.T * sc)[128 * j:128 * (j + 1),
                                128 * s:128 * (s + 1)] for W, sc in ws],
                    axis=1) for j in range(2)], axis=1))
        wo2 = _bf(np.concatenate(
            [Wo.T[128 * s:128 * (s + 1), :] * (1.0 - alpha),
             Wo.T[128 * s:128 * (s + 1), :] * alpha], axis=1))

        def _xlay(x):
            # [D, T] -> [128 p, nt, j, 512]: p = row within channel half j
            a = x.T.reshape(2, 128, T).transpose(1, 2, 0)  # [p, t, j]
            nqt = T // QT
            return _bf(a.reshape(128, nqt, QT, 2).transpose(0, 1, 3, 2))

        m = {
            "xc": _xlay(content[b]),
            "xk": _xlay(category[b]),
            "wqkc": wcols([(Wqc, scale_q), (Wkc, 1.0)]),
            "wv": wcols([(Wv, 1.0)]),
            "wqkk": wcols([(Wqk, scale_q), (Wkk, 1.0)]),
            "wo2": wo2,
            "bo_r": (_bf(bo[None, :]) if s == 0
                     else np.zeros((1, 256), ml_dtypes.bfloat16)),
            "mzro": mzro,
            "id128": _bf(np.eye(128, dtype=np.float32)),
        }
        in_maps.append(m)
    return in_maps


def _check_mask(mask, T):
    exp = np.triu(np.ones((T, T), dtype=bool), k=1)
    return np.array_equal(np.asarray(mask), exp)


def run(inputs, T=2048, cores=None, causal=True, **run_kwargs):
    """Build/compile (cached), run on hardware, return BassKernelResults."""
    key = (T, causal)
    if key not in _prog_cache:
        _prog_cache[key] = _build_program(T, causal=causal)
    nc = _prog_cache[key]
    in_maps = _host_prep(inputs, T)
    if cores is None:
        cores = list(range(len(in_maps)))
    res = run_bass_kernel_spmd(nc, [in_maps[c] for c in cores],
                               core_ids=list(range(len(cores))), **run_kwargs)
    return res


def kernel(**inputs):
    T = 2048
    mask = np.asarray(inputs["causal_mask"])
    if _check_mask(mask, T):
        causal = True
    elif not mask.any():
        causal = False
    else:
        raise NotImplementedError("kernel supports causal or empty masks only")
    res = run(inputs, T=T, causal=causal)
    nb = np.asarray(inputs["content"]).shape[0]
    out = np.empty((nb, T, D), np.float32)
    for b in range(nb):
        out[b] = res.results[2 * b]["y"] + res.results[2 * b + 1]["y"]
    return out
